# revision 4
# baseline (speedup 1.0000x reference)
"""Trainium2 Bass kernel for the BQNN boson-sampling simulation, v2.

Pure data parallel over 8 NeuronCores (batch 32768 -> 8 x 4096); per core
batch maps to [128 partitions x 32 free slots] (t). Angles are computed in
"turns" so range reduction is a truncating int round-trip, and sin/cos come
from the Activation engine's Sin table (domain [-pi,pi], 4-ULP) via its free
scale/bias affine: sin(2*pi*frac - pi). The 10-MZI ansatz runs as 4 layer
instructions groups (L1, C1, L2, C2) on a REIM-paired V layout with 4-dim
access patterns so each instruction covers all rotations of a layer. The
permanent stage computes a full 6x6 outer-product grid (rectangular APs),
symmetrizes into 15 pair values, gathers the 20-config operand tiles, and
normalizes with a strided tensor_reduce + Act sqrt.
"""

import math
import numpy as np

import concourse.bass as bass
import concourse.mybir as mybir
from concourse.tile import TileContext

F32 = mybir.dt.float32
I32 = mybir.dt.int32
ALU = mybir.AluOpType
ACTF = mybir.ActivationFunctionType

N_CORES = 8
BATCH = 32768
SHARD = BATCH // N_CORES          # 4096
P = 128
T = 32
TWO_PI = 2.0 * math.pi
TURN_SHIFT = 8.0                  # keeps y = angle/2pi + shift in [4, 13]

ANSATZ_MODES = [(0, 1), (2, 3), (4, 5), (1, 2), (3, 4)] * 2
PAIRS = [(j, k) for j in range(6) for k in range(j + 1, 6)]
PAIR_IDX = {p: i for i, p in enumerate(PAIRS)}
TRIPLES = [(i, j, k) for i in range(6) for j in range(i + 1, 6)
           for k in range(j + 1, 6)]


def _clements_modes(n=6):
    pairs = []
    for layer in range(n):
        start = 0 if layer % 2 == 0 else 1
        for m in range(start, n - 1, 2):
            pairs.append((m, m + 1))
    return pairs


CLEMENTS_MODES = _clements_modes(6)

# ---- cst row layout (in blocks of 32 floats) ----
BLK_K2 = 0          # 12 blocks: k[j]/2pi
BLK_B2 = 12         # 24 blocks: [b/2pi+shift | b/2pi+0.25+shift]
BLK_CC = 36         # 20 blocks: per C-layer {ar2,ai2,nai2,st2,ct2} x2 rots
BLK_VI = 56         # 36 blocks: S3 init, REIM row layout
CST_BLKS = 92


def _calc_start_cols(params, output_phase):
    phi = np.asarray(params[0:15], dtype=np.float32)
    theta = np.asarray(params[15:30], dtype=np.float32)
    U = np.eye(6, dtype=np.complex64)
    for k, (m, n) in enumerate(CLEMENTS_MODES):
        ct = np.complex64(np.cos(theta[k], dtype=np.float32))
        st = np.complex64(np.sin(theta[k], dtype=np.float32))
        ep = np.exp(1j * np.complex64(phi[k]))
        Tm = np.eye(6, dtype=np.complex64)
        Tm[m, m] = ep * ct
        Tm[m, n] = -st
        Tm[n, m] = ep * st
        Tm[n, n] = ct
        U = Tm @ U
    D = np.diag(np.exp(1j * np.asarray(output_phase, dtype=np.float32)
                       .astype(np.complex64)))
    return (D @ U)[:, 0:3]


def build_cst_row(params, output_phase, param_phi, param_theta,
                  input_k, input_b):
    row = np.zeros(CST_BLKS * T, dtype=np.float32)

    def setb(b, vals):
        row[b * T:(b + 1) * T] = vals

    k = np.asarray(input_k, np.float32)
    b = np.asarray(input_b, np.float32)
    for j in range(12):
        setb(BLK_K2 + j, np.float32(k[j] / TWO_PI))
        setb(BLK_B2 + j, np.float32(b[j] / TWO_PI + TURN_SHIFT))
        setb(BLK_B2 + 12 + j, np.float32(b[j] / TWO_PI + 0.25 + TURN_SHIFT))
    # C-layer coefficient tiles: rot slots (3,4) then (8,9)
    for cl, rots in enumerate([(3, 4), (8, 9)]):
        base = BLK_CC + 10 * cl
        for q, kk in enumerate(rots):
            slot = {3: 0, 4: 1, 8: 2, 9: 3}[kk]
            ph = np.float32(param_phi[slot])
            th = np.float32(param_theta[slot])
            ct = np.float32(np.cos(th)); st = np.float32(np.sin(th))
            cp = np.float32(np.cos(ph)); sp = np.float32(np.sin(ph))
            setb(base + 0 + q, cp)             # phase cos
            setb(base + 2 + q, sp)             # phase sin
            setb(base + 4 + q, -sp)            # neg phase sin
            setb(base + 6 + q, st)             # st
            setb(base + 8 + q, ct)             # ct
    s3 = _calc_start_cols(params, output_phase)
    for m in range(6):
        for c in range(3):
            setb(BLK_VI + 6 * m + c, np.real(s3[m, c]))
            setb(BLK_VI + 6 * m + 3 + c, np.imag(s3[m, c]))
    return row


# ------------------------------------------------------------- AP helpers

def _ap(tile, base_blk, dims):
    a = tile[:, 0:1]
    return bass.AP(a.tensor, a.offset + base_blk * T, [a.ap[0]] + dims)


def vap(tile, base_blk, rot_stride_blk, n, nb):
    """[p][rot: n][blk: nb][elem: 32] view of V-ish tile."""
    return _ap(tile, base_blk, [[rot_stride_blk * T, n], [T, nb], [1, T]])


def aap(tile, base_blk, rot_stride_blk, n, nb):
    """angle operand: per-rot block broadcast over nb blocks."""
    return _ap(tile, base_blk, [[rot_stride_blk * T, n], [0, nb], [1, T]])


def blk(tile, start, nblk):
    return _ap(tile, start, [[T, nblk], [1, T]])


def bcast(tile, b, nblk):
    return _ap(tile, b, [[0, nblk], [1, T]])


def row_strided(tile, base_blk, nblk):
    return _ap(tile, base_blk, [[6 * T, nblk], [1, T]])


# ------------------------------------------------------------- build

def build_kernel(reps=1, debug=False):
    nc = bass.Bass()
    xv_ext = nc.declare_dram_parameter("xv", [P, 12 * T], F32, isOutput=False)
    cst_ext = nc.declare_dram_parameter("cst", [P, CST_BLKS * T], F32,
                                        isOutput=False)
    out_ext = nc.declare_dram_parameter("out", [SHARD, 20], F32, isOutput=True)
    dbg = {}
    if debug:
        for nm, w in [("dSC", 24 * T), ("dV1", 36 * T), ("dVF", 36 * T),
                      ("dPR", 15 * T), ("dPI", 15 * T), ("dAR", 20 * T),
                      ("dAI", 20 * T)]:
            dbg[nm] = nc.declare_dram_parameter(nm, [P, w], F32, isOutput=True)

    with TileContext(nc) as tc:
        with tc.tile_pool(name="st", bufs=1) as stp, \
             tc.tile_pool(name="ang", bufs=2) as angp, \
             tc.tile_pool(name="big", bufs=1) as bigp:
            V = nc.vector
            G = nc.gpsimd
            A = nc.scalar

            CST = stp.tile([P, CST_BLKS * T], F32, name="CST", tag="CST")
            XV = stp.tile([P, 12 * T], F32, name="XV", tag="XV")
            nc.sync.dma_start(out=CST[:, :], in_=cst_ext[:, :])
            nc.sync.dma_start(out=XV[:, :], in_=xv_ext[:, :])

            # ---------------- front-half chunks (chain-heavy) --------------
            def emit_angles(cx):
                AFF = angp.tile([P, 24 * T], F32, name="AFF", tag="AFF")
                x2 = _ap(XV, 0, [[0, 2], [1, 12 * T]])
                k2 = _ap(CST, BLK_K2, [[0, 2], [1, 12 * T]])
                b2 = _ap(CST, BLK_B2, [[1, 24 * T]])
                V.tensor_tensor(_ap(AFF, 0, [[12 * T, 2], [1, 12 * T]]),
                                x2, k2, ALU.mult)
                V.tensor_tensor(AFF[:, :], AFF[:, :], b2, ALU.add)
                YI = angp.tile([P, 24 * T], I32, name="YI", tag="YI")
                V.tensor_copy(YI[:, :], AFF[:, :])
                YF = angp.tile([P, 24 * T], F32, name="YF", tag="YF")
                V.tensor_copy(YF[:, :], YI[:, :])
                FR = angp.tile([P, 24 * T], F32, name="FR", tag="FR")
                V.tensor_tensor(FR[:, :], AFF[:, :], YF[:, :], ALU.subtract)
                SC = angp.tile([P, 24 * T], F32, name="SC", tag="SC")
                A.activation(SC[:, :], FR[:, :], ACTF.Sin, bias=0.0,
                             scale=TWO_PI)
                if debug:
                    nc.sync.dma_start(out=dbg["dSC"][:, :], in_=SC[:, :])
                NSP = angp.tile([P, 6 * T], F32, name="NSP", tag="NSP")
                V.tensor_scalar(_ap(NSP, 0, [[3 * T, 2], [1, 3 * T]]),
                                _ap(SC, 0, [[6 * T, 2], [1, 3 * T]]),
                                -1.0, None, ALU.mult)
                cx["SC"] = SC
                cx["NSP"] = NSP
                cx["VV"] = angp.tile([P, 36 * T], F32, name="VV", tag="VV")
                cx["WT"] = angp.tile([P, 18 * T], F32, name="WT", tag="WT")
                cx["SW"] = angp.tile([P, 18 * T], F32, name="SW", tag="SW")
                cx["G1"] = angp.tile([P, 18 * T], F32, name="G1", tag="G1")
                cx["G2"] = angp.tile([P, 18 * T], F32, name="G2", tag="G2")

            def layer(cx, srcT, src_base, n, cp, sp, nsp, ct, st):
                VV, WT, SW = cx["VV"], cx["WT"], cx["SW"]
                G1, G2 = cx["G1"], cx["G2"]

                def ang(spec, nb):
                    t, bb, rs = spec
                    return aap(t, bb, rs, n, nb)
                vm6 = vap(srcT, src_base, 12, n, 6)
                vmr = vap(srcT, src_base, 12, n, 3)
                vmi = vap(srcT, src_base + 3, 12, n, 3)
                vn6 = vap(srcT, src_base + 6, 12, n, 6)
                wt6 = vap(WT, 0, 6, n, 6)
                swr = vap(SW, 0, 6, n, 3)
                swi = vap(SW, 3, 6, n, 3)
                sw6 = vap(SW, 0, 6, n, 6)
                g16 = vap(G1, 0, 6, n, 6)
                g26 = vap(G2, 0, 6, n, 6)
                dvm = vap(VV, 0, 12, n, 6)
                dvn = vap(VV, 6, 12, n, 6)
                V.tensor_tensor(wt6, ang(cp, 6), vm6, ALU.mult)
                G.tensor_tensor(swr, ang(nsp, 3), vmi, ALU.mult)
                G.tensor_tensor(swi, ang(sp, 3), vmr, ALU.mult)
                V.tensor_tensor(wt6, wt6, sw6, ALU.add)
                V.tensor_tensor(g16, ang(ct, 6), wt6, ALU.mult)
                G.tensor_tensor(g26, ang(st, 6), vn6, ALU.mult)
                V.tensor_tensor(dvm, g16, g26, ALU.subtract)
                V.tensor_tensor(g16, ang(st, 6), wt6, ALU.mult)
                G.tensor_tensor(g26, ang(ct, 6), vn6, ALU.mult)
                V.tensor_tensor(dvn, g16, g26, ALU.add)

            def clayer(cx, base_cc, vbase):
                VV, WT, SW = cx["VV"], cx["WT"], cx["SW"]
                G1, G2 = cx["G1"], cx["G2"]

                def ang2(off, nb):
                    return aap(CST, base_cc + off, 1, 2, nb)
                vm6 = vap(VV, vbase, 12, 2, 6)
                vmr = vap(VV, vbase, 12, 2, 3)
                vmi = vap(VV, vbase + 3, 12, 2, 3)
                vn6 = vap(VV, vbase + 6, 12, 2, 6)
                wt6 = vap(WT, 0, 6, 2, 6)
                swr = vap(SW, 0, 6, 2, 3)
                swi = vap(SW, 3, 6, 2, 3)
                sw6 = vap(SW, 0, 6, 2, 6)
                g16 = vap(G1, 0, 6, 2, 6)
                g26 = vap(G2, 0, 6, 2, 6)
                V.tensor_tensor(wt6, ang2(0, 6), vm6, ALU.mult)
                G.tensor_tensor(swr, ang2(4, 3), vmi, ALU.mult)
                G.tensor_tensor(swi, ang2(2, 3), vmr, ALU.mult)
                V.tensor_tensor(wt6, wt6, sw6, ALU.add)
                V.tensor_tensor(g16, ang2(8, 6), wt6, ALU.mult)
                G.tensor_tensor(g26, ang2(6, 6), vn6, ALU.mult)
                V.tensor_tensor(vm6, g16, g26, ALU.subtract)
                V.tensor_tensor(g16, ang2(6, 6), wt6, ALU.mult)
                G.tensor_tensor(g26, ang2(8, 6), vn6, ALU.mult)
                V.tensor_tensor(vn6, g16, g26, ALU.add)

            def emit_L1(cx):
                SC, NSP = cx["SC"], cx["NSP"]
                layer(cx, CST, BLK_VI, 3,
                      cp=(SC, 12, 1), sp=(SC, 0, 1), nsp=(NSP, 0, 1),
                      ct=(SC, 15, 1), st=(SC, 3, 1))
                if debug:
                    nc.sync.dma_start(out=dbg["dV1"][:, :], in_=cx["VV"][:, :])

            def emit_C1(cx):
                clayer(cx, BLK_CC, 6)

            def emit_L2(cx):
                SC, NSP = cx["SC"], cx["NSP"]
                layer(cx, cx["VV"], 0, 3,
                      cp=(SC, 18, 1), sp=(SC, 6, 1), nsp=(NSP, 3, 1),
                      ct=(SC, 21, 1), st=(SC, 9, 1))

            def emit_C2(cx):
                clayer(cx, BLK_CC + 10, 6)
                if debug:
                    nc.sync.dma_start(out=dbg["dVF"][:, :], in_=cx["VV"][:, :])

            # ---------------- back-half chunks (throughput-heavy) ----------
            def emit_P(cx):
                VV = cx["VV"]
                GRR = bigp.tile([P, 36 * T], F32, name="GRR", tag="GRR")
                GII = bigp.tile([P, 36 * T], F32, name="GII", tag="GII")
                GRI = bigp.tile([P, 36 * T], F32, name="GRI", tag="GRI")
                GIR = bigp.tile([P, 36 * T], F32, name="GIR", tag="GIR")
                yr = _ap(VV, 1, [[6 * T, 6], [0, 6], [1, T]])
                yi = _ap(VV, 4, [[6 * T, 6], [0, 6], [1, T]])
                zr = _ap(VV, 2, [[0, 6], [6 * T, 6], [1, T]])
                zi = _ap(VV, 5, [[0, 6], [6 * T, 6], [1, T]])

                def g36(tile):
                    return _ap(tile, 0, [[6 * T, 6], [T, 6], [1, T]])

                V.tensor_tensor(g36(GRR), yr, zr, ALU.mult)
                V.tensor_tensor(g36(GII), yi, zi, ALU.mult)
                G.tensor_tensor(g36(GRI), yr, zi, ALU.mult)
                G.tensor_tensor(g36(GIR), yi, zr, ALU.mult)
                DT = bigp.tile([P, 36 * T], F32, name="DT", tag="DT")
                ET = bigp.tile([P, 36 * T], F32, name="ET", tag="ET")
                V.tensor_tensor(DT[:, :], GRR[:, :], GII[:, :], ALU.subtract)
                G.tensor_tensor(ET[:, :], GRI[:, :], GIR[:, :], ALU.add)
                cx["DT"] = DT
                cx["ET"] = ET

            def emit_PrPi(cx):
                DT, ET = cx["DT"], cx["ET"]
                PR = bigp.tile([P, 15 * T], F32, name="PR", tag="PR")
                PI = bigp.tile([P, 15 * T], F32, name="PI", tag="PI")
                s = 0
                for j in range(5):
                    L = 5 - j
                    up = _ap(DT, 6 * j + j + 1, [[1, L * T]])
                    lo = _ap(DT, 6 * (j + 1) + j, [[6 * T, L], [1, T]])
                    V.tensor_tensor(blk(PR, s, L), up, lo, ALU.add)
                    upe = _ap(ET, 6 * j + j + 1, [[1, L * T]])
                    loe = _ap(ET, 6 * (j + 1) + j, [[6 * T, L], [1, T]])
                    V.tensor_tensor(blk(PI, s, L), upe, loe, ALU.add)
                    s += L
                cx["PR"] = PR
                cx["PI"] = PI
                if debug:
                    nc.sync.dma_start(out=dbg["dPR"][:, :], in_=PR[:, :])
                    nc.sync.dma_start(out=dbg["dPI"][:, :], in_=PI[:, :])

            def emit_gathers_a(cx):
                VV, PR, PI = cx["VV"], cx["PR"], cx["PI"]
                X1 = [bigp.tile([P, 20 * T], F32, name=f"X1{c}", tag=f"X1{c}")
                      for c in range(2)]
                X2 = [bigp.tile([P, 20 * T], F32, name=f"X2{c}", tag=f"X2{c}")
                      for c in range(2)]
                PA = [bigp.tile([P, 20 * T], F32, name=f"PA{c}", tag=f"PA{c}")
                      for c in range(2)]
                s = 0
                for i in range(4):
                    L = (5 - i) * (4 - i) // 2
                    for c in range(2):
                        A.copy(blk(X1[c], s, L), bcast(VV, 6 * i + 3 * c, L))
                        pt = PR if c == 0 else PI
                        A.copy(blk(PA[c], s, L),
                               _ap(pt, PAIR_IDX[(i + 1, i + 2)], [[1, L * T]]))
                    s += L
                s = 0
                for i in range(4):
                    for j in range(i + 1, 5):
                        L = 5 - j
                        for c in range(2):
                            A.copy(blk(X2[c], s, L),
                                   bcast(VV, 6 * j + 3 * c, L))
                        s += L
                cx.update(X1=X1, X2=X2, PA=PA)

            def emit_gathers_b(cx):
                VV, PR, PI = cx["VV"], cx["PR"], cx["PI"]
                X3 = [bigp.tile([P, 20 * T], F32, name=f"X3{c}", tag=f"X3{c}")
                      for c in range(2)]
                PB = [bigp.tile([P, 20 * T], F32, name=f"PB{c}", tag=f"PB{c}")
                      for c in range(2)]
                PC = [bigp.tile([P, 20 * T], F32, name=f"PC{c}", tag=f"PC{c}")
                      for c in range(2)]
                s = 0
                for i in range(4):
                    for j in range(i + 1, 5):
                        L = 5 - j
                        for c in range(2):
                            pt = PR if c == 0 else PI
                            G.tensor_copy(blk(X3[c], s, L),
                                          row_strided(VV, 6 * (j + 1) + 3 * c,
                                                      L))
                            V.tensor_copy(blk(PB[c], s, L),
                                          _ap(pt, PAIR_IDX[(i, j + 1)],
                                              [[1, L * T]]))
                            G.tensor_copy(blk(PC[c], s, L),
                                          bcast(pt, PAIR_IDX[(i, j)], L))
                        s += L
                cx.update(X3=X3, PB=PB, PC=PC)

            def emit_products(cx):
                X1, X2, X3 = cx["X1"], cx["X2"], cx["X3"]
                PA, PB, PC = cx["PA"], cx["PB"], cx["PC"]
                W20 = 20 * T
                PRD = [bigp.tile([P, W20], F32, name=f"PRD{q}", tag=f"PRD{q}")
                       for q in range(12)]
                V.tensor_tensor(PRD[0][:, :], X1[0][:, :], PA[0][:, :], ALU.mult)
                G.tensor_tensor(PRD[1][:, :], X1[1][:, :], PA[1][:, :], ALU.mult)
                V.tensor_tensor(PRD[2][:, :], X2[0][:, :], PB[0][:, :], ALU.mult)
                G.tensor_tensor(PRD[3][:, :], X2[1][:, :], PB[1][:, :], ALU.mult)
                V.tensor_tensor(PRD[4][:, :], X3[0][:, :], PC[0][:, :], ALU.mult)
                G.tensor_tensor(PRD[5][:, :], X3[1][:, :], PC[1][:, :], ALU.mult)
                V.tensor_tensor(PRD[6][:, :], X1[0][:, :], PA[1][:, :], ALU.mult)
                G.tensor_tensor(PRD[7][:, :], X1[1][:, :], PA[0][:, :], ALU.mult)
                V.tensor_tensor(PRD[8][:, :], X2[0][:, :], PB[1][:, :], ALU.mult)
                G.tensor_tensor(PRD[9][:, :], X2[1][:, :], PB[0][:, :], ALU.mult)
                V.tensor_tensor(PRD[10][:, :], X3[0][:, :], PC[1][:, :], ALU.mult)
                G.tensor_tensor(PRD[11][:, :], X3[1][:, :], PC[0][:, :], ALU.mult)
                cx["PRD"] = PRD

            def emit_tail1(cx):
                PRD = cx["PRD"]
                W20 = 20 * T
                AR = bigp.tile([P, W20], F32, name="AR", tag="AR")
                t1 = bigp.tile([P, W20], F32, name="t1", tag="t1")
                V.tensor_tensor(t1[:, :], PRD[0][:, :], PRD[1][:, :],
                                ALU.subtract)
                V.tensor_tensor(t1[:, :], t1[:, :], PRD[2][:, :], ALU.add)
                V.tensor_tensor(t1[:, :], t1[:, :], PRD[3][:, :], ALU.subtract)
                V.tensor_tensor(t1[:, :], t1[:, :], PRD[4][:, :], ALU.add)
                V.tensor_tensor(AR[:, :], t1[:, :], PRD[5][:, :], ALU.subtract)
                AI = bigp.tile([P, W20], F32, name="AI", tag="AI")
                V.tensor_tensor(t1[:, :], PRD[6][:, :], PRD[7][:, :], ALU.add)
                V.tensor_tensor(t1[:, :], t1[:, :], PRD[8][:, :], ALU.add)
                V.tensor_tensor(t1[:, :], t1[:, :], PRD[9][:, :], ALU.add)
                V.tensor_tensor(t1[:, :], t1[:, :], PRD[10][:, :], ALU.add)
                V.tensor_tensor(AI[:, :], t1[:, :], PRD[11][:, :], ALU.add)
                if debug:
                    nc.sync.dma_start(out=dbg["dAR"][:, :], in_=AR[:, :])
                    nc.sync.dma_start(out=dbg["dAI"][:, :], in_=AI[:, :])
                cx["AR"] = AR
                cx["AI"] = AI
                cx["t1"] = t1

            def emit_tail2(cx):
                AR, AI, t1 = cx["AR"], cx["AI"], cx["t1"]
                W20 = 20 * T
                AB = bigp.tile([P, 21 * T], F32, name="AB", tag="AB")
                SQ1 = bigp.tile([P, W20], F32, name="SQ1", tag="SQ1")
                A.activation(SQ1[:, :], AR[:, :], ACTF.Square)
                A.activation(t1[:, :], AI[:, :], ACTF.Square)
                V.tensor_tensor(AB[:, 0:W20], SQ1[:, :], t1[:, :], ALU.add)
                V.tensor_reduce(AB[:, W20:21 * T],
                                _ap(AB, 0, [[1, T], [T, 20]]),
                                mybir.AxisListType.X, ALU.add)
                SQ = bigp.tile([P, 21 * T], F32, name="SQ", tag="SQ")
                A.activation(SQ[:, :], AB[:, :], ACTF.Sqrt)
                RINV = bigp.tile([P, T], F32, name="RINV", tag="RINV")
                V.tensor_scalar_max(SQ[:, W20:21 * T], SQ[:, W20:21 * T],
                                    1e-12)
                V.reciprocal(RINV[:, :], SQ[:, W20:21 * T])
                OUT2 = bigp.tile([P, W20], F32, name="OUT2", tag="OUT2")
                o2 = OUT2[:, 0:1]
                out_tc = bass.AP(o2.tensor, o2.offset, [o2.ap[0], [1, 20],
                                                        [20, T]])
                V.tensor_tensor(out_tc, blk(SQ, 0, 20),
                                _ap(RINV, 0, [[0, 20], [1, T]]), ALU.mult)
                oa = out_ext[:, :]
                dst = bass.AP(oa.tensor, 0, [[20 * T, P], [1, 20 * T]])
                nc.sync.dma_start(out=dst, in_=OUT2[:, :])

            # ------- software-pipelined emission: front(r) ¦ back(r-1) -----
            def emit_back(cx):
                emit_P(cx)
                emit_PrPi(cx)
                emit_gathers_a(cx)
                emit_gathers_b(cx)
                emit_products(cx)
                emit_tail1(cx)
                emit_tail2(cx)

            pend = None
            for _rep in range(reps):
                cx = {}
                emit_angles(cx)
                if pend is not None:
                    emit_P(pend)
                emit_L1(cx)
                if pend is not None:
                    emit_PrPi(pend)
                emit_C1(cx)
                if pend is not None:
                    emit_gathers_a(pend)
                    emit_gathers_b(pend)
                emit_L2(cx)
                if pend is not None:
                    emit_products(pend)
                emit_C2(cx)
                if pend is not None:
                    emit_tail1(pend)
                    emit_tail2(pend)
                pend = cx
            emit_back(pend)

    _split_excess_waits(nc)
    return nc


def _split_excess_waits(nc):
    """HW compute instructions hold at most 1 embedded sem-wait; hoist extras
    onto EventSemaphore instructions (cap 2 each)."""
    nsplit = 0
    for f in nc.m.functions:
        for blkk in f.blocks:
            new = []
            for inst in blkk.instructions:
                si = inst.sync_info
                if (si is not None and len(si.on_wait) > 1
                        and type(inst).__name__ != "InstEventSemaphore"):
                    waits = list(si.on_wait)
                    keep, extra = waits[-1], waits[:-1]
                    while extra:
                        chunk, extra = extra[:2], extra[2:]
                        nsplit += 1
                        new.append(mybir.InstEventSemaphore(
                            name=f"{inst.name}-ws{nsplit}",
                            engine=inst.engine, ins=[], outs=[],
                            sync_info=mybir.SyncInfo(on_wait=chunk,
                                                     on_update=[])))
                    inst.sync_info = mybir.SyncInfo(
                        on_wait=[keep], on_update=list(si.on_update))
                new.append(inst)
            blkk.instructions = new


_NC_CACHE = {}


def build_in_maps(x, params, output_phase, param_phi, param_theta,
                  input_k, input_b):
    x = np.ascontiguousarray(np.asarray(x, dtype=np.float32))
    row = build_cst_row(params, output_phase, param_phi, param_theta,
                        input_k, input_b)
    cst = np.tile(row, (P, 1)).astype(np.float32)
    in_maps = []
    for i in range(N_CORES):
        shard = x[i * SHARD:(i + 1) * SHARD].reshape(P, T, 12)
        xv = np.ascontiguousarray(shard.transpose(0, 2, 1).reshape(P, 12 * T))
        in_maps.append({"xv": xv, "cst": cst})
    return in_maps


def _make_callable(nc, n_cores=N_CORES):
    import jax
    from jax.sharding import Mesh, PartitionSpec
    from jax.experimental.shard_map import shard_map
    from concourse.bass2jax import (install_neuronx_cc_hook, _bass_exec_p,
                                    partition_id_tensor)
    install_neuronx_cc_hook()
    in_names, out_names, out_avals, zero_outs = [], [], [], []
    for alloc in nc.m.functions[0].allocations:
        if not isinstance(alloc, mybir.MemoryLocationSet):
            continue
        name = alloc.memorylocations[0].name
        if alloc.kind == "ExternalInput":
            if name != "partition_id":
                in_names.append(name)
        elif alloc.kind == "ExternalOutput":
            out_names.append(name)
            shape = tuple(alloc.tensor_shape)
            dtype = mybir.dt.np(alloc.dtype)
            out_avals.append(jax.core.ShapedArray(shape, dtype))
            zero_outs.append(np.zeros(shape, dtype))
    n_params = len(in_names)
    n_outs = len(out_avals)
    has_pid = nc.partition_id_tensor is not None
    all_in = in_names + out_names + (["partition_id"] if has_pid else [])

    def _body(*args):
        operands = list(args)
        if has_pid:
            operands.append(partition_id_tensor())
        outs = _bass_exec_p.bind(
            *operands, out_avals=tuple(out_avals), in_names=tuple(all_in),
            out_names=tuple(out_names), lowering_input_output_aliases=(),
            sim_require_finite=True, sim_require_nnan=True, nc=nc)
        return tuple(outs)

    import jax
    devices = jax.devices()[:n_cores]
    mesh = Mesh(np.asarray(devices), ("core",))
    f = jax.jit(shard_map(_body, mesh=mesh,
                in_specs=(PartitionSpec("core"),) * (n_params + n_outs),
                out_specs=(PartitionSpec("core"),) * n_outs, check_rep=False),
                keep_unused=True)
    return f, in_names, zero_outs


def kernel(x, params, output_phase, param_phi, param_theta, input_k, input_b):
    if "f" not in _NC_CACHE:
        nc = build_kernel()
        _NC_CACHE["nc"] = nc
        _NC_CACHE["f"] = _make_callable(nc)
    f, in_names, zero_outs = _NC_CACHE["f"]
    in_maps = build_in_maps(x, params, output_phase, param_phi, param_theta,
                            input_k, input_b)
    gin = [np.concatenate([in_maps[c][n] for c in range(N_CORES)], axis=0)
           for n in in_names]
    gz = [np.zeros((N_CORES * z.shape[0], *z.shape[1:]), z.dtype)
          for z in zero_outs]
    out_arr = np.asarray(f(*(gin + gz))[0])
    return np.ascontiguousarray(out_arr.reshape(BATCH, 20)).astype(np.float32)


# revision 5
# speedup vs baseline: 1.1799x; 1.1799x over previous
"""Trainium2 Bass kernel for the BQNN boson-sampling simulation, v2.

Pure data parallel over 8 NeuronCores (batch 32768 -> 8 x 4096); per core
batch maps to [128 partitions x 32 free slots] (t). Angles are computed in
"turns" so range reduction is a truncating int round-trip, and sin/cos come
from the Activation engine's Sin table (domain [-pi,pi], 4-ULP) via its free
scale/bias affine: sin(2*pi*frac - pi). The 10-MZI ansatz runs as 4 layer
instructions groups (L1, C1, L2, C2) on a REIM-paired V layout with 4-dim
access patterns so each instruction covers all rotations of a layer. The
permanent stage computes a full 6x6 outer-product grid (rectangular APs),
symmetrizes into 15 pair values, gathers the 20-config operand tiles, and
normalizes with a strided tensor_reduce + Act sqrt.
"""

import math
import numpy as np

import concourse.bass as bass
import concourse.mybir as mybir
from concourse.tile import TileContext

F32 = mybir.dt.float32
I32 = mybir.dt.int32
ALU = mybir.AluOpType
ACTF = mybir.ActivationFunctionType

N_CORES = 8
BATCH = 32768
SHARD = BATCH // N_CORES          # 4096
P = 128
T = 32
TWO_PI = 2.0 * math.pi
TURN_SHIFT = 8.0                  # keeps y = angle/2pi + shift in [4, 13]

ANSATZ_MODES = [(0, 1), (2, 3), (4, 5), (1, 2), (3, 4)] * 2
PAIRS = [(j, k) for j in range(6) for k in range(j + 1, 6)]
PAIR_IDX = {p: i for i, p in enumerate(PAIRS)}
TRIPLES = [(i, j, k) for i in range(6) for j in range(i + 1, 6)
           for k in range(j + 1, 6)]


def _clements_modes(n=6):
    pairs = []
    for layer in range(n):
        start = 0 if layer % 2 == 0 else 1
        for m in range(start, n - 1, 2):
            pairs.append((m, m + 1))
    return pairs


CLEMENTS_MODES = _clements_modes(6)

# ---- cst row layout (in blocks of 32 floats) ----
BLK_K2 = 0          # 12 blocks: k[j]/2pi
BLK_B2 = 12         # 24 blocks: [b/2pi+shift | b/2pi+0.25+shift]
BLK_CC = 36         # 20 blocks: per C-layer {ar2,ai2,nai2,st2,ct2} x2 rots
BLK_VI = 56         # 36 blocks: S3 init, REIM row layout
CST_BLKS = 92


def _calc_start_cols(params, output_phase):
    phi = np.asarray(params[0:15], dtype=np.float32)
    theta = np.asarray(params[15:30], dtype=np.float32)
    U = np.eye(6, dtype=np.complex64)
    for k, (m, n) in enumerate(CLEMENTS_MODES):
        ct = np.complex64(np.cos(theta[k], dtype=np.float32))
        st = np.complex64(np.sin(theta[k], dtype=np.float32))
        ep = np.exp(1j * np.complex64(phi[k]))
        Tm = np.eye(6, dtype=np.complex64)
        Tm[m, m] = ep * ct
        Tm[m, n] = -st
        Tm[n, m] = ep * st
        Tm[n, n] = ct
        U = Tm @ U
    D = np.diag(np.exp(1j * np.asarray(output_phase, dtype=np.float32)
                       .astype(np.complex64)))
    return (D @ U)[:, 0:3]


def build_cst_row(params, output_phase, param_phi, param_theta,
                  input_k, input_b):
    row = np.zeros(CST_BLKS * T, dtype=np.float32)

    def setb(b, vals):
        row[b * T:(b + 1) * T] = vals

    k = np.asarray(input_k, np.float32)
    b = np.asarray(input_b, np.float32)
    for j in range(12):
        setb(BLK_K2 + j, np.float32(k[j] / TWO_PI))
        setb(BLK_B2 + j, np.float32(b[j] / TWO_PI + TURN_SHIFT))
        setb(BLK_B2 + 12 + j, np.float32(b[j] / TWO_PI + 0.25 + TURN_SHIFT))
    # C-layer coefficient tiles: rot slots (3,4) then (8,9)
    for cl, rots in enumerate([(3, 4), (8, 9)]):
        base = BLK_CC + 10 * cl
        for q, kk in enumerate(rots):
            slot = {3: 0, 4: 1, 8: 2, 9: 3}[kk]
            ph = np.float32(param_phi[slot])
            th = np.float32(param_theta[slot])
            ct = np.float32(np.cos(th)); st = np.float32(np.sin(th))
            cp = np.float32(np.cos(ph)); sp = np.float32(np.sin(ph))
            setb(base + 0 + q, cp)             # phase cos
            setb(base + 2 + q, sp)             # phase sin
            setb(base + 4 + q, -sp)            # neg phase sin
            setb(base + 6 + q, st)             # st
            setb(base + 8 + q, ct)             # ct
    s3 = _calc_start_cols(params, output_phase)
    for m in range(6):
        for c in range(3):
            setb(BLK_VI + 6 * m + c, np.real(s3[m, c]))
            setb(BLK_VI + 6 * m + 3 + c, np.imag(s3[m, c]))
    return row


# ------------------------------------------------------------- AP helpers

def _ap(tile, base_blk, dims):
    a = tile[:, 0:1]
    return bass.AP(a.tensor, a.offset + base_blk * T, [a.ap[0]] + dims)


def vap(tile, base_blk, rot_stride_blk, n, nb):
    """[p][rot: n][blk: nb][elem: 32] view of V-ish tile."""
    return _ap(tile, base_blk, [[rot_stride_blk * T, n], [T, nb], [1, T]])


def aap(tile, base_blk, rot_stride_blk, n, nb):
    """angle operand: per-rot block broadcast over nb blocks."""
    return _ap(tile, base_blk, [[rot_stride_blk * T, n], [0, nb], [1, T]])


def blk(tile, start, nblk):
    return _ap(tile, start, [[T, nblk], [1, T]])


def bcast(tile, b, nblk):
    return _ap(tile, b, [[0, nblk], [1, T]])


def row_strided(tile, base_blk, nblk):
    return _ap(tile, base_blk, [[6 * T, nblk], [1, T]])


# ------------------------------------------------------------- build

def build_kernel(reps=1, debug=False):
    nc = bass.Bass()
    xv_ext = nc.declare_dram_parameter("xv", [P, 12 * T], F32, isOutput=False)
    cst_ext = nc.declare_dram_parameter("cst", [P, CST_BLKS * T], F32,
                                        isOutput=False)
    out_ext = nc.declare_dram_parameter("out", [SHARD, 20], F32, isOutput=True)
    dbg = {}
    if debug:
        for nm, w in [("dSC", 24 * T), ("dV1", 36 * T), ("dVF", 36 * T),
                      ("dPR", 15 * T), ("dPI", 15 * T), ("dAR", 20 * T),
                      ("dAI", 20 * T)]:
            dbg[nm] = nc.declare_dram_parameter(nm, [P, w], F32, isOutput=True)

    with TileContext(nc) as tc:
        with tc.tile_pool(name="st", bufs=1) as stp, \
             tc.tile_pool(name="ang", bufs=2) as angp, \
             tc.tile_pool(name="big", bufs=1) as bigp:
            V = nc.vector
            G = nc.gpsimd
            A = nc.scalar

            CST = stp.tile([P, CST_BLKS * T], F32, name="CST", tag="CST")
            XV = stp.tile([P, 12 * T], F32, name="XV", tag="XV")
            nc.sync.dma_start(out=CST[:, :], in_=cst_ext[:, :])
            nc.sync.dma_start(out=XV[:, :], in_=xv_ext[:, :])

            # ---------------- front-half chunks (chain-heavy) --------------
            def emit_angles(cx):
                AFF = angp.tile([P, 24 * T], F32, name="AFF", tag="AFF")
                x2 = _ap(XV, 0, [[0, 2], [1, 12 * T]])
                k2 = _ap(CST, BLK_K2, [[0, 2], [1, 12 * T]])
                b2 = _ap(CST, BLK_B2, [[1, 24 * T]])
                V.tensor_tensor(_ap(AFF, 0, [[12 * T, 2], [1, 12 * T]]),
                                x2, k2, ALU.mult)
                V.tensor_tensor(AFF[:, :], AFF[:, :], b2, ALU.add)
                YI = angp.tile([P, 24 * T], I32, name="YI", tag="YI")
                V.tensor_copy(YI[:, :], AFF[:, :])
                YF = angp.tile([P, 24 * T], F32, name="YF", tag="YF")
                V.tensor_copy(YF[:, :], YI[:, :])
                FR = angp.tile([P, 24 * T], F32, name="FR", tag="FR")
                V.tensor_tensor(FR[:, :], AFF[:, :], YF[:, :], ALU.subtract)
                SC = angp.tile([P, 24 * T], F32, name="SC", tag="SC")
                A.activation(SC[:, :], FR[:, :], ACTF.Sin, bias=0.0,
                             scale=TWO_PI)
                if debug:
                    nc.sync.dma_start(out=dbg["dSC"][:, :], in_=SC[:, :])
                NSP = angp.tile([P, 6 * T], F32, name="NSP", tag="NSP")
                V.tensor_scalar(_ap(NSP, 0, [[3 * T, 2], [1, 3 * T]]),
                                _ap(SC, 0, [[6 * T, 2], [1, 3 * T]]),
                                -1.0, None, ALU.mult)
                cx["SC"] = SC
                cx["NSP"] = NSP
                cx["VV"] = angp.tile([P, 36 * T], F32, name="VV", tag="VV")
                cx["WT"] = angp.tile([P, 18 * T], F32, name="WT", tag="WT")
                cx["SW"] = angp.tile([P, 18 * T], F32, name="SW", tag="SW")
                cx["G1"] = angp.tile([P, 18 * T], F32, name="G1", tag="G1")
                cx["G2"] = angp.tile([P, 18 * T], F32, name="G2", tag="G2")

            def layer(cx, srcT, src_base, n, cp, sp, nsp, ct, st):
                VV, WT, SW = cx["VV"], cx["WT"], cx["SW"]
                G1, G2 = cx["G1"], cx["G2"]

                def ang(spec, nb):
                    t, bb, rs = spec
                    return aap(t, bb, rs, n, nb)
                vm6 = vap(srcT, src_base, 12, n, 6)
                vmr = vap(srcT, src_base, 12, n, 3)
                vmi = vap(srcT, src_base + 3, 12, n, 3)
                vn6 = vap(srcT, src_base + 6, 12, n, 6)
                wt6 = vap(WT, 0, 6, n, 6)
                swr = vap(SW, 0, 6, n, 3)
                swi = vap(SW, 3, 6, n, 3)
                sw6 = vap(SW, 0, 6, n, 6)
                g16 = vap(G1, 0, 6, n, 6)
                g26 = vap(G2, 0, 6, n, 6)
                dvm = vap(VV, 0, 12, n, 6)
                dvn = vap(VV, 6, 12, n, 6)
                V.tensor_tensor(wt6, ang(cp, 6), vm6, ALU.mult)
                G.tensor_tensor(swr, ang(nsp, 3), vmi, ALU.mult)
                G.tensor_tensor(swi, ang(sp, 3), vmr, ALU.mult)
                V.tensor_tensor(wt6, wt6, sw6, ALU.add)
                V.tensor_tensor(g16, ang(ct, 6), wt6, ALU.mult)
                G.tensor_tensor(g26, ang(st, 6), vn6, ALU.mult)
                V.tensor_tensor(dvm, g16, g26, ALU.subtract)
                V.tensor_tensor(g16, ang(st, 6), wt6, ALU.mult)
                G.tensor_tensor(g26, ang(ct, 6), vn6, ALU.mult)
                V.tensor_tensor(dvn, g16, g26, ALU.add)

            def clayer(cx, base_cc, vbase):
                VV, WT, SW = cx["VV"], cx["WT"], cx["SW"]
                G1, G2 = cx["G1"], cx["G2"]

                def ang2(off, nb):
                    return aap(CST, base_cc + off, 1, 2, nb)
                vm6 = vap(VV, vbase, 12, 2, 6)
                vmr = vap(VV, vbase, 12, 2, 3)
                vmi = vap(VV, vbase + 3, 12, 2, 3)
                vn6 = vap(VV, vbase + 6, 12, 2, 6)
                wt6 = vap(WT, 0, 6, 2, 6)
                swr = vap(SW, 0, 6, 2, 3)
                swi = vap(SW, 3, 6, 2, 3)
                sw6 = vap(SW, 0, 6, 2, 6)
                g16 = vap(G1, 0, 6, 2, 6)
                g26 = vap(G2, 0, 6, 2, 6)
                V.tensor_tensor(wt6, ang2(0, 6), vm6, ALU.mult)
                G.tensor_tensor(swr, ang2(4, 3), vmi, ALU.mult)
                G.tensor_tensor(swi, ang2(2, 3), vmr, ALU.mult)
                V.tensor_tensor(wt6, wt6, sw6, ALU.add)
                V.tensor_tensor(g16, ang2(8, 6), wt6, ALU.mult)
                G.tensor_tensor(g26, ang2(6, 6), vn6, ALU.mult)
                V.tensor_tensor(vm6, g16, g26, ALU.subtract)
                V.tensor_tensor(g16, ang2(6, 6), wt6, ALU.mult)
                G.tensor_tensor(g26, ang2(8, 6), vn6, ALU.mult)
                V.tensor_tensor(vn6, g16, g26, ALU.add)

            def emit_L1(cx):
                SC, NSP = cx["SC"], cx["NSP"]
                layer(cx, CST, BLK_VI, 3,
                      cp=(SC, 12, 1), sp=(SC, 0, 1), nsp=(NSP, 0, 1),
                      ct=(SC, 15, 1), st=(SC, 3, 1))
                if debug:
                    nc.sync.dma_start(out=dbg["dV1"][:, :], in_=cx["VV"][:, :])

            def emit_C1(cx):
                clayer(cx, BLK_CC, 6)

            def emit_L2(cx):
                SC, NSP = cx["SC"], cx["NSP"]
                layer(cx, cx["VV"], 0, 3,
                      cp=(SC, 18, 1), sp=(SC, 6, 1), nsp=(NSP, 3, 1),
                      ct=(SC, 21, 1), st=(SC, 9, 1))

            def emit_C2(cx):
                clayer(cx, BLK_CC + 10, 6)
                if debug:
                    nc.sync.dma_start(out=dbg["dVF"][:, :], in_=cx["VV"][:, :])

            # ---------------- back-half chunks (throughput-heavy) ----------
            def emit_P(cx):
                VV = cx["VV"]
                GRR = bigp.tile([P, 36 * T], F32, name="GRR", tag="GRR")
                GII = bigp.tile([P, 36 * T], F32, name="GII", tag="GII")
                GRI = bigp.tile([P, 36 * T], F32, name="GRI", tag="GRI")
                GIR = bigp.tile([P, 36 * T], F32, name="GIR", tag="GIR")
                yr = _ap(VV, 1, [[6 * T, 6], [0, 6], [1, T]])
                yi = _ap(VV, 4, [[6 * T, 6], [0, 6], [1, T]])
                zr = _ap(VV, 2, [[0, 6], [6 * T, 6], [1, T]])
                zi = _ap(VV, 5, [[0, 6], [6 * T, 6], [1, T]])

                def g36(tile):
                    return _ap(tile, 0, [[6 * T, 6], [T, 6], [1, T]])

                V.tensor_tensor(g36(GRR), yr, zr, ALU.mult)
                V.tensor_tensor(g36(GII), yi, zi, ALU.mult)
                G.tensor_tensor(g36(GRI), yr, zi, ALU.mult)
                G.tensor_tensor(g36(GIR), yi, zr, ALU.mult)
                DT = bigp.tile([P, 36 * T], F32, name="DT", tag="DT")
                ET = bigp.tile([P, 36 * T], F32, name="ET", tag="ET")
                V.tensor_tensor(DT[:, :], GRR[:, :], GII[:, :], ALU.subtract)
                G.tensor_tensor(ET[:, :], GRI[:, :], GIR[:, :], ALU.add)
                cx["DT"] = DT
                cx["ET"] = ET

            def emit_PrPi(cx):
                DT, ET = cx["DT"], cx["ET"]
                PR = bigp.tile([P, 15 * T], F32, name="PR", tag="PR")
                PI = bigp.tile([P, 15 * T], F32, name="PI", tag="PI")
                s = 0
                for j in range(5):
                    L = 5 - j
                    up = _ap(DT, 6 * j + j + 1, [[1, L * T]])
                    lo = _ap(DT, 6 * (j + 1) + j, [[6 * T, L], [1, T]])
                    V.tensor_tensor(blk(PR, s, L), up, lo, ALU.add)
                    upe = _ap(ET, 6 * j + j + 1, [[1, L * T]])
                    loe = _ap(ET, 6 * (j + 1) + j, [[6 * T, L], [1, T]])
                    V.tensor_tensor(blk(PI, s, L), upe, loe, ALU.add)
                    s += L
                cx["PR"] = PR
                cx["PI"] = PI
                if debug:
                    nc.sync.dma_start(out=dbg["dPR"][:, :], in_=PR[:, :])
                    nc.sync.dma_start(out=dbg["dPI"][:, :], in_=PI[:, :])

            def emit_gathers_a(cx):
                VV, PR, PI = cx["VV"], cx["PR"], cx["PI"]
                X1 = [bigp.tile([P, 20 * T], F32, name=f"X1{c}", tag=f"X1{c}")
                      for c in range(2)]
                X2 = [bigp.tile([P, 20 * T], F32, name=f"X2{c}", tag=f"X2{c}")
                      for c in range(2)]
                PA = [bigp.tile([P, 20 * T], F32, name=f"PA{c}", tag=f"PA{c}")
                      for c in range(2)]
                s = 0
                for i in range(4):
                    L = (5 - i) * (4 - i) // 2
                    for c in range(2):
                        A.copy(blk(X1[c], s, L), bcast(VV, 6 * i + 3 * c, L))
                        pt = PR if c == 0 else PI
                        A.copy(blk(PA[c], s, L),
                               _ap(pt, PAIR_IDX[(i + 1, i + 2)], [[1, L * T]]))
                    s += L
                s = 0
                for i in range(4):
                    for j in range(i + 1, 5):
                        L = 5 - j
                        for c in range(2):
                            V.tensor_copy(blk(X2[c], s, L),
                                          bcast(VV, 6 * j + 3 * c, L))
                        s += L
                cx.update(X1=X1, X2=X2, PA=PA)

            def emit_gathers_b(cx):
                VV, PR, PI = cx["VV"], cx["PR"], cx["PI"]
                X3 = [bigp.tile([P, 20 * T], F32, name=f"X3{c}", tag=f"X3{c}")
                      for c in range(2)]
                PB = [bigp.tile([P, 20 * T], F32, name=f"PB{c}", tag=f"PB{c}")
                      for c in range(2)]
                PC = [bigp.tile([P, 20 * T], F32, name=f"PC{c}", tag=f"PC{c}")
                      for c in range(2)]
                s = 0
                for i in range(4):
                    for j in range(i + 1, 5):
                        L = 5 - j
                        for c in range(2):
                            pt = PR if c == 0 else PI
                            G.tensor_copy(blk(X3[c], s, L),
                                          row_strided(VV, 6 * (j + 1) + 3 * c,
                                                      L))
                            V.tensor_copy(blk(PB[c], s, L),
                                          _ap(pt, PAIR_IDX[(i, j + 1)],
                                              [[1, L * T]]))
                            G.tensor_copy(blk(PC[c], s, L),
                                          bcast(pt, PAIR_IDX[(i, j)], L))
                        s += L
                cx.update(X3=X3, PB=PB, PC=PC)

            def emit_products(cx):
                X1, X2, X3 = cx["X1"], cx["X2"], cx["X3"]
                PA, PB, PC = cx["PA"], cx["PB"], cx["PC"]
                W20 = 20 * T
                PRD = [bigp.tile([P, W20], F32, name=f"PRD{q}", tag=f"PRD{q}")
                       for q in range(12)]
                V.tensor_tensor(PRD[0][:, :], X1[0][:, :], PA[0][:, :], ALU.mult)
                G.tensor_tensor(PRD[1][:, :], X1[1][:, :], PA[1][:, :], ALU.mult)
                V.tensor_tensor(PRD[2][:, :], X2[0][:, :], PB[0][:, :], ALU.mult)
                G.tensor_tensor(PRD[3][:, :], X2[1][:, :], PB[1][:, :], ALU.mult)
                V.tensor_tensor(PRD[4][:, :], X3[0][:, :], PC[0][:, :], ALU.mult)
                G.tensor_tensor(PRD[5][:, :], X3[1][:, :], PC[1][:, :], ALU.mult)
                V.tensor_tensor(PRD[6][:, :], X1[0][:, :], PA[1][:, :], ALU.mult)
                G.tensor_tensor(PRD[7][:, :], X1[1][:, :], PA[0][:, :], ALU.mult)
                V.tensor_tensor(PRD[8][:, :], X2[0][:, :], PB[1][:, :], ALU.mult)
                G.tensor_tensor(PRD[9][:, :], X2[1][:, :], PB[0][:, :], ALU.mult)
                V.tensor_tensor(PRD[10][:, :], X3[0][:, :], PC[1][:, :], ALU.mult)
                G.tensor_tensor(PRD[11][:, :], X3[1][:, :], PC[0][:, :], ALU.mult)
                cx["PRD"] = PRD

            def emit_tail1(cx):
                PRD = cx["PRD"]
                W20 = 20 * T
                AR = bigp.tile([P, W20], F32, name="AR", tag="AR")
                t1 = bigp.tile([P, W20], F32, name="t1", tag="t1")
                V.tensor_tensor(t1[:, :], PRD[0][:, :], PRD[1][:, :],
                                ALU.subtract)
                V.tensor_tensor(t1[:, :], t1[:, :], PRD[2][:, :], ALU.add)
                V.tensor_tensor(t1[:, :], t1[:, :], PRD[3][:, :], ALU.subtract)
                V.tensor_tensor(t1[:, :], t1[:, :], PRD[4][:, :], ALU.add)
                V.tensor_tensor(AR[:, :], t1[:, :], PRD[5][:, :], ALU.subtract)
                AI = bigp.tile([P, W20], F32, name="AI", tag="AI")
                V.tensor_tensor(t1[:, :], PRD[6][:, :], PRD[7][:, :], ALU.add)
                V.tensor_tensor(t1[:, :], t1[:, :], PRD[8][:, :], ALU.add)
                V.tensor_tensor(t1[:, :], t1[:, :], PRD[9][:, :], ALU.add)
                V.tensor_tensor(t1[:, :], t1[:, :], PRD[10][:, :], ALU.add)
                V.tensor_tensor(AI[:, :], t1[:, :], PRD[11][:, :], ALU.add)
                if debug:
                    nc.sync.dma_start(out=dbg["dAR"][:, :], in_=AR[:, :])
                    nc.sync.dma_start(out=dbg["dAI"][:, :], in_=AI[:, :])
                cx["AR"] = AR
                cx["AI"] = AI
                cx["t1"] = t1

            def emit_tail2(cx):
                AR, AI, t1 = cx["AR"], cx["AI"], cx["t1"]
                W20 = 20 * T
                AB = bigp.tile([P, 21 * T], F32, name="AB", tag="AB")
                SQ1 = bigp.tile([P, W20], F32, name="SQ1", tag="SQ1")
                A.activation(SQ1[:, :], AR[:, :], ACTF.Square)
                A.activation(t1[:, :], AI[:, :], ACTF.Square)
                V.tensor_tensor(AB[:, 0:W20], SQ1[:, :], t1[:, :], ALU.add)
                V.tensor_reduce(AB[:, W20:21 * T],
                                _ap(AB, 0, [[1, T], [T, 20]]),
                                mybir.AxisListType.X, ALU.add)
                SQ = bigp.tile([P, 21 * T], F32, name="SQ", tag="SQ")
                A.activation(SQ[:, :], AB[:, :], ACTF.Sqrt)
                RINV = bigp.tile([P, T], F32, name="RINV", tag="RINV")
                V.tensor_scalar_max(SQ[:, W20:21 * T], SQ[:, W20:21 * T],
                                    1e-12)
                V.reciprocal(RINV[:, :], SQ[:, W20:21 * T])
                OUT2 = bigp.tile([P, W20], F32, name="OUT2", tag="OUT2")
                o2 = OUT2[:, 0:1]
                out_tc = bass.AP(o2.tensor, o2.offset, [o2.ap[0], [1, 20],
                                                        [20, T]])
                V.tensor_tensor(out_tc, blk(SQ, 0, 20),
                                _ap(RINV, 0, [[0, 20], [1, T]]), ALU.mult)
                oa = out_ext[:, :]
                dst = bass.AP(oa.tensor, 0, [[20 * T, P], [1, 20 * T]])
                nc.sync.dma_start(out=dst, in_=OUT2[:, :])

            # ------- software-pipelined emission: front(r) ¦ back(r-1) -----
            def emit_back(cx):
                emit_P(cx)
                emit_PrPi(cx)
                emit_gathers_a(cx)
                emit_gathers_b(cx)
                emit_products(cx)
                emit_tail1(cx)
                emit_tail2(cx)

            pend = None
            for _rep in range(reps):
                cx = {}
                emit_angles(cx)
                if pend is not None:
                    emit_P(pend)
                emit_L1(cx)
                if pend is not None:
                    emit_PrPi(pend)
                emit_C1(cx)
                if pend is not None:
                    emit_gathers_a(pend)
                    emit_gathers_b(pend)
                emit_L2(cx)
                if pend is not None:
                    emit_products(pend)
                emit_C2(cx)
                if pend is not None:
                    emit_tail1(pend)
                    emit_tail2(pend)
                pend = cx
            emit_back(pend)

    _split_excess_waits(nc)
    return nc


def _split_excess_waits(nc):
    """HW compute instructions hold at most 1 embedded sem-wait; hoist extras
    onto EventSemaphore instructions (cap 2 each)."""
    nsplit = 0
    for f in nc.m.functions:
        for blkk in f.blocks:
            new = []
            for inst in blkk.instructions:
                si = inst.sync_info
                if (si is not None and len(si.on_wait) > 1
                        and type(inst).__name__ != "InstEventSemaphore"):
                    waits = list(si.on_wait)
                    keep, extra = waits[-1], waits[:-1]
                    while extra:
                        chunk, extra = extra[:2], extra[2:]
                        nsplit += 1
                        new.append(mybir.InstEventSemaphore(
                            name=f"{inst.name}-ws{nsplit}",
                            engine=inst.engine, ins=[], outs=[],
                            sync_info=mybir.SyncInfo(on_wait=chunk,
                                                     on_update=[])))
                    inst.sync_info = mybir.SyncInfo(
                        on_wait=[keep], on_update=list(si.on_update))
                new.append(inst)
            blkk.instructions = new


_NC_CACHE = {}


def build_in_maps(x, params, output_phase, param_phi, param_theta,
                  input_k, input_b):
    x = np.ascontiguousarray(np.asarray(x, dtype=np.float32))
    row = build_cst_row(params, output_phase, param_phi, param_theta,
                        input_k, input_b)
    cst = np.tile(row, (P, 1)).astype(np.float32)
    in_maps = []
    for i in range(N_CORES):
        shard = x[i * SHARD:(i + 1) * SHARD].reshape(P, T, 12)
        xv = np.ascontiguousarray(shard.transpose(0, 2, 1).reshape(P, 12 * T))
        in_maps.append({"xv": xv, "cst": cst})
    return in_maps


def _make_callable(nc, n_cores=N_CORES):
    import jax
    from jax.sharding import Mesh, PartitionSpec
    from jax.experimental.shard_map import shard_map
    from concourse.bass2jax import (install_neuronx_cc_hook, _bass_exec_p,
                                    partition_id_tensor)
    install_neuronx_cc_hook()
    in_names, out_names, out_avals, zero_outs = [], [], [], []
    for alloc in nc.m.functions[0].allocations:
        if not isinstance(alloc, mybir.MemoryLocationSet):
            continue
        name = alloc.memorylocations[0].name
        if alloc.kind == "ExternalInput":
            if name != "partition_id":
                in_names.append(name)
        elif alloc.kind == "ExternalOutput":
            out_names.append(name)
            shape = tuple(alloc.tensor_shape)
            dtype = mybir.dt.np(alloc.dtype)
            out_avals.append(jax.core.ShapedArray(shape, dtype))
            zero_outs.append(np.zeros(shape, dtype))
    n_params = len(in_names)
    n_outs = len(out_avals)
    has_pid = nc.partition_id_tensor is not None
    all_in = in_names + out_names + (["partition_id"] if has_pid else [])

    def _body(*args):
        operands = list(args)
        if has_pid:
            operands.append(partition_id_tensor())
        outs = _bass_exec_p.bind(
            *operands, out_avals=tuple(out_avals), in_names=tuple(all_in),
            out_names=tuple(out_names), lowering_input_output_aliases=(),
            sim_require_finite=True, sim_require_nnan=True, nc=nc)
        return tuple(outs)

    import jax
    devices = jax.devices()[:n_cores]
    mesh = Mesh(np.asarray(devices), ("core",))
    f = jax.jit(shard_map(_body, mesh=mesh,
                in_specs=(PartitionSpec("core"),) * (n_params + n_outs),
                out_specs=(PartitionSpec("core"),) * n_outs, check_rep=False),
                keep_unused=True)
    return f, in_names, zero_outs


def kernel(x, params, output_phase, param_phi, param_theta, input_k, input_b):
    if "f" not in _NC_CACHE:
        nc = build_kernel()
        _NC_CACHE["nc"] = nc
        _NC_CACHE["f"] = _make_callable(nc)
    f, in_names, zero_outs = _NC_CACHE["f"]
    in_maps = build_in_maps(x, params, output_phase, param_phi, param_theta,
                            input_k, input_b)
    gin = [np.concatenate([in_maps[c][n] for c in range(N_CORES)], axis=0)
           for n in in_names]
    gz = [np.zeros((N_CORES * z.shape[0], *z.shape[1:]), z.dtype)
          for z in zero_outs]
    out_arr = np.asarray(f(*(gin + gz))[0])
    return np.ascontiguousarray(out_arr.reshape(BATCH, 20)).astype(np.float32)
